# revision 13
# baseline (speedup 1.0000x reference)
"""Trainium2 kernel for the bilinear form y[b,k] = sum_ij x[b,i] x[b,j] W[i,j,k] + b[k].

Shapes: x (512, 784) f32, W (614656=784*784, 10) f32, b (10,) f32 -> y (512, 10) f32.

Strategy (8 NeuronCores):
  - Shard the j axis of W.reshape(784, 784, 10) across cores: 98 j's per core.
    Each core reads W/8 + full x (~2.5 MB in fp16); compute is the long pole.
  - Stage 1 (TensorE): U[b, (k,j)] = sum_i x[b,i] * W[i, j_shard, k], x^T tiles
    stationary, W shard moving, accumulating over 7 uniform 112-row i-tiles
    into 8 PSUM banks (4 batch tiles x 2 column halves = 5 k's x 98 j's).
  - Stage 2: y[b, k] = sum_j U[b, (k,j)] * x[b, j]: multiply on DVE (PSUM
    path) or Act-copy + Pool (SBUF path), then reduce over j on DVE.
  - Host: y = sum_c y_part_c + b  (20 KB per core; no collectives needed).

Schedule notes (arrival-driven):
  - W is h-split: h0 chunks ride the sync HWDGE ring, h1 the scalar ring,
    interleaved with x^T chunks in first-consumption order; per-ring FIFO
    plus shared physical DMA engines make arrival track global issue order.
  - Matmuls: warmups (no DMA deps, warm the HAM clock gate), i-tile-major
    prefix chasing chunk arrivals (h0 its 0-2, h1 its 0-1), then per-group
    finish runs so the 8 PSUM groups stop staggered ~0.8 us apart and
    stage 2 (DVE/Pool/Act) overlaps the tail of the matmul stream.
  - Pool-path groups run early (their Act-copy+Pool+DVE chain is ~2.3 us),
    DVE-path groups late (~1.2 us chain), minimizing the post-matmul tail.
  - y leaves in two 10 KB DMAs (h0 mid-kernel, h1 at the end).
"""

import numpy as np

D = 784
B = 512
C = 10
NCORES = 8
JS = D // NCORES  # 98 j's per core
JK = JS * C  # 980 free columns per core, laid out as (k, j)
HALF = JK // 2  # 490 = 5 k's x 98 j's -> one PSUM bank
KH = C // 2  # 5 k's per half
P = 128
B_TILES = B // P  # 4
IT = 7  # i-tiles
IP = D // IT  # 112 rows per i-tile (uniform, no padding)
N_WARMUP_MM = 8  # dummy matmuls (N=HALF) that warm the PE clock gate

MM_DTYPE = "float16"  # dtype of the matmul operands (and their DMA)

_nc_cache = {}


def _build_nc():
    import concourse.bacc as bacc
    import concourse.mybir as mybir
    import concourse.tile as tile

    mm_dt = getattr(mybir.dt, MM_DTYPE)
    f32 = mybir.dt.float32

    nc = bacc.Bacc("TRN2", target_bir_lowering=False)

    # Partition-major DRAM layouts (see _make_in_maps).
    xT = nc.dram_tensor("xT", [IP, IT, B], mm_dt, kind="ExternalInput")
    w = nc.dram_tensor("w", [IP, 2, IT, HALF], mm_dt, kind="ExternalInput")
    xs = nc.dram_tensor("xs", [P, B_TILES, JS], f32, kind="ExternalInput")
    y = nc.dram_tensor("y", [P, 2, B_TILES, KH], f32, kind="ExternalOutput")

    with tile.TileContext(nc) as tc:
        with (
            tc.tile_pool(name="wpool", bufs=8) as wpool,
            tc.tile_pool(name="xpool", bufs=3) as xpool,
            tc.tile_pool(name="xspool", bufs=1) as xspool,
            tc.tile_pool(name="ypool", bufs=1) as ypool,
            tc.tile_pool(name="scratch", bufs=10) as spool,
            tc.tile_pool(name="psum", bufs=8, space="PSUM") as psum_pool,
        ):
            w_sb = {}  # (it, h) -> [IP, HALF] view

            def w_dma(eng, h, c0, c1):
                wt = wpool.tile(
                    [IP, c1 - c0, HALF], mm_dt, name=f"w_h{h}c{c0}", tag=f"wh{h}"
                )
                eng.dma_start(wt[:], w[:, h, c0:c1, :])
                for it in range(c0, c1):
                    w_sb[(it, h)] = wt[:, it - c0, :]

            xT_sb = {}

            def xt_dma(c0, c1):
                xt = xpool.tile([IP, c1 - c0, B], mm_dt, name=f"xt_c{c0}", tag="xt")
                nc.scalar.dma_start(xt[:], xT[:, c0:c1, :])
                for it in range(c0, c1):
                    xT_sb[it] = xt[:, it - c0, :]

            # Issue order == first-consumption order (per ring). Chunks span
            # 2 i-tiles (1.96 KB/partition rows): per-DMA cost is descriptor-
            # count-bound (112/chunk), so wider chunks ~double early
            # delivery bandwidth.
            # sync:   wh0[0:2], wh0[2:4], wh0[4:6], wh0[6], y...
            # scalar: xt[0:2], wh1[0:2], xs, xt[2:4], wh1[2:4], xt[4:7],
            #         wh1[4:6], wh1[6]
            w_dma(nc.sync, 0, 0, 2)
            xt_dma(0, 2)
            w_dma(nc.scalar, 1, 0, 2)
            w_dma(nc.sync, 0, 2, 4)
            xt_dma(2, 4)
            w_dma(nc.sync, 0, 4, 6)
            w_dma(nc.scalar, 1, 2, 4)
            w_dma(nc.sync, 0, 6, 7)
            xs_sb = xspool.tile([P, B_TILES, JS], f32)
            nc.scalar.dma_start(xs_sb[:], xs[:])
            xt_dma(4, 7)
            w_dma(nc.scalar, 1, 4, 6)
            w_dma(nc.scalar, 1, 6, 7)

            # PSUM: 8 accumulation groups (bt, h), one bank each. Warmups
            # write into group (0,0)'s bank; the first real start=True matmul
            # clears has_written so the garbage is discarded.
            pts = {}
            for bt in range(B_TILES):
                for h in range(2):
                    pts[(bt, h)] = psum_pool.tile(
                        [P, HALF], f32, name=f"pt_b{bt}h{h}", tag="pt", bufs=8
                    )

            dmy_s = spool.tile([IP, P], mm_dt, name="dmy_s", tag="dmy_s", bufs=1)
            dmy_m = spool.tile([IP, HALF], mm_dt, name="dmy_m", tag="dmy_m", bufs=1)
            nc.gpsimd.memset(dmy_s[:], 0.0)
            nc.gpsimd.memset(dmy_m[:], 0.0)
            for _ in range(N_WARMUP_MM):
                nc.tensor.matmul(
                    pts[(0, 0)][:], dmy_s[:], dmy_m[:], start=True, stop=True
                )

            y_t = ypool.tile([P, 2, B_TILES, KH], f32)

            def mm(it, bt, h, start, stop):
                nc.tensor.matmul(
                    pts[(bt, h)][:],
                    xT_sb[it][:, bt * P : (bt + 1) * P],
                    w_sb[(it, h)][:],
                    start=start,
                    stop=stop,
                )

            def stage2(bt, h, on_dve):
                pt = pts[(bt, h)]
                scr = spool.tile(
                    [P, HALF], f32, name=f"scr{bt}{h}", tag="scr", bufs=4
                )
                s3 = scr[:].rearrange("p (kh j) -> p kh j", kh=KH)
                xs3 = xs_sb[:, bt, None, :].broadcast_to([P, KH, JS])
                if on_dve:
                    p3 = pt[:].rearrange("p (kh j) -> p kh j", kh=KH)
                    nc.vector.tensor_tensor(s3, p3, xs3, mybir.AluOpType.mult)
                else:
                    ucopy = spool.tile(
                        [P, HALF], f32, name=f"uc{bt}{h}", tag="ucopy", bufs=2
                    )
                    nc.scalar.activation(
                        ucopy[:], pt[:], mybir.ActivationFunctionType.Copy
                    )
                    u3 = ucopy[:].rearrange("p (kh j) -> p kh j", kh=KH)
                    nc.gpsimd.tensor_tensor(s3, u3, xs3, mybir.AluOpType.mult)
                nc.vector.tensor_reduce(
                    out=y_t[:, h, bt, :],
                    in_=s3,
                    op=mybir.AluOpType.add,
                    axis=mybir.AxisListType.X,
                )

            # Prefix, i-tile-major, chasing chunk arrivals (sync delivers
            # wh0 chunk pairs, scalar wh1; consumption alternates to match).
            for it, h in ((0, 0), (1, 0), (0, 1), (1, 1), (2, 0), (3, 0), (2, 1)):
                for bt in range(B_TILES):
                    mm(it, bt, h, start=(it == 0), stop=False)

            # Finish phase: h0 groups (its 4-6), then h1 groups (its 3-6);
            # each group stops and immediately enters stage 2. Pool-path
            # groups first within each half, DVE-path last.
            for gi, (bt, h) in enumerate(
                [(bt, 0) for bt in range(B_TILES)] + [(bt, 1) for bt in range(B_TILES)]
            ):
                for it in range(4 - h, IT):
                    mm(it, bt, h, start=False, stop=(it == IT - 1))
                stage2(bt, h, on_dve=(bt % 2 == 1))
                if gi == 3:
                    nc.sync.dma_start(y[:, 0], y_t[:, 0])
            nc.sync.dma_start(y[:, 1], y_t[:, 1])

    nc.compile()
    return nc


def _get_nc():
    if "nc" not in _nc_cache:
        _nc_cache["nc"] = _build_nc()
    return _nc_cache["nc"]


def _make_in_maps(x, W):
    import concourse.mybir as mybir

    mm_np = mybir.dt.np(getattr(mybir.dt, MM_DTYPE))
    x = np.asarray(x, dtype=np.float32)
    Wr = np.asarray(W, dtype=np.float32).reshape(D, D, C)
    # xT_dram[p, it, b] = x[b, it*IP + p]
    xT = np.ascontiguousarray(
        x.T.astype(mm_np).reshape(IT, IP, B).transpose(1, 0, 2)
    )
    in_maps = []
    for c in range(NCORES):
        js, je = c * JS, (c + 1) * JS
        # wsh[i, k*JS + j] = Wr[i, js+j, k]; then [p, h, it, col] partition-major
        wsh = Wr[:, js:je, :].transpose(0, 2, 1).reshape(D, JK).astype(mm_np)
        wshard = np.ascontiguousarray(
            wsh.reshape(IT, IP, 2, HALF).transpose(1, 2, 0, 3)
        )
        # xs_dram[p, bt, j] = x[bt*P + p, js + j]
        xsl = np.ascontiguousarray(
            x[:, js:je].reshape(B_TILES, P, JS).transpose(1, 0, 2)
        )
        in_maps.append({"xT": xT, "w": wshard, "xs": xsl})
    return in_maps


def run_spmd(x, W, **spmd_kwargs):
    """Compile/run the SPMD kernel; returns (partials, BassKernelResults)."""
    from concourse.bass_utils import run_bass_kernel_spmd

    nc = _get_nc()
    in_maps = _make_in_maps(x, W)
    res = run_bass_kernel_spmd(nc, in_maps, core_ids=list(range(NCORES)), **spmd_kwargs)
    # y_dram[p, h, bt, kh] -> y[bt*P + p, h*KH + kh]
    partials = [
        r["y"].transpose(2, 0, 1, 3).reshape(B_TILES, P, C).reshape(B, C)
        for r in res.results
    ]
    return partials, res


def kernel(x, W, b):
    partials, _ = run_spmd(x, W)
    y = np.sum(np.stack(partials, 0), axis=0, dtype=np.float64) + np.asarray(
        b, dtype=np.float64
    )
    return y.astype(np.float32)
